# revision 18
# baseline (speedup 1.0000x reference)
"""AQT int8-quantized matmul (dynamic symmetric quantization) on 8 TRN2 cores.

Full problem: lhs [8192, 4096] f32 @ rhs [4096, 4096] f32 with per-row lhs
scales and per-column rhs scales (abs-max / 127.5), int8 round+clip, int32
matmul, dequantize by the outer product of scales.

Sharding: 2x4 grid over (M, N). Each core gets lhs rows M/2 and rhs cols N/4,
computes its [4096, 1024] output block; host assembles the 8 blocks. Both
quantization axes keep their full contraction dim on every core, so per-core
results match the unsharded reference exactly. No collectives needed.

Numerics: quantized values are exact integers in [-127, 127] stored as bf16;
TensorE matmul with fp32 PSUM accumulation reproduces the int32 matmul.
round() is exact via the +1.5*2^23 magic-constant trick; the quant divisor is
shrunk by (1-2^-20) so no post-round clip is needed.

Schedule (v4):
- rhs quantization is DVE-centric: pass A is DMA + one tensor_tensor(abs_max)
  chain per [128,1024] k-tile; pass B re-reads rhs, u = t*r_bc (tt) and
  q = (u+C)-C as one two-op tensor_scalar, f32->bf16. ACT only does the lhs
  quant chunks, so neither engine gates the HBM-paced prologue (~110us for
  rhs 2x16MB + early lhs).
- lhs: 4x[128,1024] chunks per m-tile; per-chunk absmax reduces combine via
  [P,1] maxes; t1/q on ACT; per-chunk DMA-xbar transpose on the SP queue into
  per-chunk lhsT tiles (so matmuls depend on chunk 0's transpose only).
- Matmul ramp: m-tiles 0/1 chains start on the first quantized rhs tiles;
  m-tiles 2/3 join late consuming already-resident k-tiles in rotated order
  (a psum chain may visit k in any order), so the in-order TensorE queue
  never parks on an unready tile.
- Steady state: two 32-long psum chains per m-tile; lhs quant runs one m-tile
  ahead, loads two ahead. PSUM eviction is one fused DVE
  scalar_tensor_tensor((psum*s_l)*s_bc); output DMA goes through the GpSimd
  software DGE so it cannot convoy the SP load/transpose queue.
"""
import sys

if "/opt/trn_rl_repo" not in sys.path:
    sys.path.insert(0, "/opt/trn_rl_repo")

from contextlib import ExitStack

import numpy as np

from concourse import bacc, bass_isa, mybir, tile
from concourse.bass_utils import run_bass_kernel_spmd

f32 = mybir.dt.float32
bf16 = mybir.dt.bfloat16
Alu = mybir.AluOpType
Act = mybir.ActivationFunctionType

P = 128
C_MAGIC = 1.5 * 2 ** 23
QDIV = 127.5 * (1.0 - 2.0 ** -20)
INV_QDIV = 1.0 / QDIV
TINY = 1e-30

M, K, N = 8192, 4096, 4096
MG, NG = 2, 4                      # shard grid rows (M) x cols (N)
M_loc, N_loc = M // MG, N // NG    # 4096, 1024 per core
N_CORES = MG * NG

CHK = 1024                         # lhs load/quant chunk (free-dim elems)


def build_aqt(nc, M_loc, K, N_loc, W=512):
    KT, MT, NB = K // P, M_loc // P, N_loc // W
    NCHK = K // CHK
    KPC = CHK // P                 # k-tiles per chunk (8)

    lhs = nc.declare_dram_parameter("lhs", [M_loc, K], f32, isOutput=False)
    rhs = nc.declare_dram_parameter("rhs", [K, N_loc], f32, isOutput=False)
    out = nc.declare_dram_parameter("out", [M_loc, N_loc], f32, isOutput=True)

    with tile.TileContext(nc) as tc, ExitStack() as ctx:
        pool = lambda name, bufs, **kw: ctx.enter_context(
            tc.tile_pool(name=name, bufs=bufs, **kw))
        qr_pool = pool("qr", KT)            # quantized rhs [P,N_loc] bf16, resident
        sbc_pool = pool("sbc", 1)           # rhs dequant scales [P,N_loc] f32
        rstage = pool("rstage", 2)          # rhs raw pass A [P,N_loc] f32
        rstage2 = pool("rstage2", 2)        # rhs raw pass B
        upool = pool("u", 1)                # rhs * r_bc
        racc = pool("racc", 2)              # max accumulator ping-pong
        rnacc = pool("rnacc", 2)            # min accumulator ping-pong
        rbc = pool("rbc", 2)                # amax_bc / r_bc
        lraw = pool("lraw", 6)              # lhs raw chunk [P, CHK] f32
        lt1 = pool("lt1", 1)                # lhs scaled+C chunk [P, CHK] f32
        lqc = pool("lqc", 2)                # lhs quantized chunk [P, CHK] bf16
        lqt = pool("lqt", 16)            # lhsT chunk tiles [P, KPC, P] bf16
        lsc = pool("lsc", 1)                # s_l columns, resident
        lam = pool("lam", 8)                # [P, 1] scratch
        opool = pool("o1", 2)
        psum = ctx.enter_context(tc.tile_pool(name="psum", bufs=8, space="PSUM"))

        s_l_all = lsc.tile([P, MT], f32)

        raw_tiles = {}                      # (mi, c) -> raw chunk
        rfirst = [None]
        amc_tiles = {}                      # (mi, c) -> [P,1] chunk absmax
        rl_tiles = {}
        qt_tiles = {}                       # mi -> [chunk tiles]
        qr_tiles = {}
        racc_state = [None, None]           # max chain, min chain
        sbc_t = [None]

        # ---------------- lhs helpers ----------------
        def lhs_load(mi, c):
            raw = lraw.tile([P, CHK], f32, name="lraw")
            nc.sync.dma_start(raw[:], lhs[mi * P:(mi + 1) * P,
                                          c * CHK:(c + 1) * CHK])
            raw_tiles[(mi, c)] = raw

        def lhs_load_all(mi):
            for c in range(NCHK):
                lhs_load(mi, c)

        def lhs_reduce(mi):
            for c in range(NCHK):
                am = lam.tile([P, 1], f32, name="lam")
                nc.vector.tensor_reduce(am[:], raw_tiles[(mi, c)][:],
                                        axis=mybir.AxisListType.X,
                                        op=Alu.max, apply_absolute_value=True)
                amc_tiles[(mi, c)] = am
            acc = amc_tiles[(mi, 0)]
            for c in range(1, NCHK):
                nacc = lam.tile([P, 1], f32, name="lam2")
                nc.vector.tensor_tensor(nacc[:], acc[:], amc_tiles[(mi, c)][:],
                                        op=Alu.max)
                acc = nacc
            s_col = s_l_all[:, mi:mi + 1]
            nc.vector.tensor_scalar(s_col, acc[:], TINY, INV_QDIV,
                                    op0=Alu.max, op1=Alu.mult)
            r_l = lam.tile([P, 1], f32, name="rl")
            nc.vector.reciprocal(r_l[:], s_col)
            rl_tiles[mi] = r_l

        def lhs_chunk(mi, c, dve_qc=False):
            if c == 0:
                qt_tiles[mi] = []
            qtc = lqt.tile([P, KPC, P], bf16, name="lqt")
            qt_tiles[mi].append(qtc)
            raw = raw_tiles.pop((mi, c))
            t1 = lt1.tile([P, CHK], f32, name="lt1")
            nc.scalar.activation(t1[:], raw[:], Act.Copy,
                                 bias=C_MAGIC, scale=rl_tiles[mi][:])
            qc = lqc.tile([P, CHK], bf16, name="lqc")
            if dve_qc:
                nc.vector.tensor_scalar(qc[:], t1[:], C_MAGIC, None,
                                        op0=Alu.subtract)
            else:
                nc.scalar.activation(qc[:], t1[:], Act.Copy, bias=-C_MAGIC)
            # transpose issues from the ACT HWDGE queue: it directly follows
            # the op producing qc, so it never convoys the SP load queue
            nc.scalar.dma_start_transpose(qtc[:], qc[:])

        def lhs_quant(mi, dve_qc=False):
            lhs_reduce(mi)
            for c in range(NCHK):
                lhs_chunk(mi, c, dve_qc)

        # ---------------- rhs helpers ----------------
        def rhs_A(kt):
            t = rstage.tile([P, N_loc], f32, name="rstage")
            nc.sync.dma_start(t[:], rhs[kt * P:(kt + 1) * P, :])
            if kt == 0:
                rfirst[0] = t
                return
            prev_mx = racc_state[0] or rfirst[0]
            prev_mn = racc_state[1] or rfirst[0]
            mx = racc.tile([P, N_loc], f32, name="racc")
            nc.vector.tensor_tensor(mx[:], prev_mx[:], t[:], op=Alu.max)
            mn = rnacc.tile([P, N_loc], f32, name="rnacc")
            nc.vector.tensor_tensor(mn[:], prev_mn[:], t[:], op=Alu.min)
            racc_state[0], racc_state[1] = mx, mn

        def rhs_scales():
            negmn = rbc.tile([P, N_loc], f32, name="negmn")
            nc.vector.tensor_scalar(negmn[:], racc_state[1][:], -1.0, None,
                                    op0=Alu.mult)
            pre = rbc.tile([P, N_loc], f32, name="pre")
            nc.vector.tensor_tensor(pre[:], racc_state[0][:], negmn[:],
                                    op=Alu.max)
            amax = rbc.tile([P, N_loc], f32, name="amax")
            nc.gpsimd.partition_all_reduce(amax[:], pre[:],
                                           channels=P,
                                           reduce_op=bass_isa.ReduceOp.absmax)
            s_bc = sbc_pool.tile([P, N_loc], f32, name="sbc")
            nc.vector.tensor_scalar(s_bc[:], amax[:], TINY, INV_QDIV,
                                    op0=Alu.max, op1=Alu.mult)
            sbc_t[0] = s_bc
            r_bc = rbc.tile([P, N_loc], f32, name="rbc")
            nc.vector.reciprocal(r_bc[:], s_bc[:])
            return r_bc

        def rhs_B(kt, r_bc):
            t2 = rstage2.tile([P, N_loc], f32, name="rstage2")
            nc.sync.dma_start(t2[:], rhs[kt * P:(kt + 1) * P, :])
            u = upool.tile([P, N_loc], f32, name="u")
            nc.vector.tensor_tensor(u[:], t2[:], r_bc[:], op=Alu.mult)
            q = qr_pool.tile([P, N_loc], bf16, name="qr")
            nc.vector.tensor_scalar(q[:], u[:], C_MAGIC, C_MAGIC,
                                    op0=Alu.add, op1=Alu.subtract)
            qr_tiles[kt] = q

        # ---------------- matmul + eviction ----------------
        def evict(mi, nb, ps):
            o = opool.tile([P, W], f32, name="o1")
            nc.vector.scalar_tensor_tensor(
                o[:], ps[:], s_l_all[:, mi:mi + 1],
                sbc_t[0][:, nb * W:(nb + 1) * W],
                op0=Alu.mult, op1=Alu.mult)
            nc.gpsimd.dma_start(
                out[mi * P:(mi + 1) * P, nb * W:(nb + 1) * W], o[:])

        def mm(ps, mi, nb, kt, start, stop):
            nc.tensor.matmul(ps[:], qt_tiles[mi][kt // KPC][:, kt % KPC, :],
                             qr_tiles[kt][:, nb * W:(nb + 1) * W],
                             start=start, stop=stop)

        def chain(mi, nb):
            ps = psum.tile([P, W], f32, name="ps")
            for kt in range(KT):
                mm(ps, mi, nb, kt, kt == 0, kt == KT - 1)
            evict(mi, nb, ps)

        def ramp(join):
            # staggered-join lockstep: chain (mi, nb) starts at clock join[mi]
            # consuming k-tiles in arrival order from its join point
            last = max(join.values()) + KT
            pss = {}
            for c in range(last):
                for mi, j0 in join.items():
                    j = c - j0
                    if not (0 <= j < KT):
                        continue
                    for nb in range(NB):
                        if j == 0:
                            pss[(mi, nb)] = psum.tile([P, W], f32, name="ps")
                        mm(pss[(mi, nb)], mi, nb, j, j == 0, j == KT - 1)
                    if j == KT - 1:
                        for nb in range(NB):
                            evict(mi, nb, pss.pop((mi, nb)))

        # ---------------- emission ----------------
        # prologue: rhs pass A paces at DMA speed; lhs m-tiles 0/1 prep under it
        lhs_load_all(0)
        lhs_load_all(1)
        for kt in range(KT):
            rhs_A(kt)
            if kt == 8:
                lhs_reduce(0)
            elif 10 <= kt < 10 + NCHK:
                lhs_chunk(0, kt - 10)
            elif kt == 28:
                lhs_load_all(2)
        lhs_quant(1)
        r_bc = rhs_scales()
        for kt in range(KT):
            rhs_B(kt, r_bc)
            if kt == 2:
                lhs_reduce(2)
            elif 4 <= kt < 4 + NCHK:
                lhs_chunk(2, kt - 4)
            elif kt == 9:
                lhs_load_all(3)
            elif kt == 14:
                lhs_reduce(3)
            elif 16 <= kt < 16 + NCHK:
                lhs_chunk(3, kt - 16)
            elif kt == 21:
                lhs_load_all(4)
            elif kt == 25:
                lhs_reduce(4)
        for c in range(NCHK):
            lhs_chunk(4, c, dve_qc=True)
        lhs_load_all(5)
        lhs_quant(5, dve_qc=True)

        # matmul ramp: m-tiles 0/1 start on the first quantized rhs tiles,
        # later m-tiles join as their lhsT tiles and psum banks free up
        ramp({0: 0, 1: 4, 2: 16, 3: 18, 4: 32, 5: 36})
        lhs_load_all(6)
        lhs_quant(6, dve_qc=True)
        lhs_load_all(7)

        # steady state: quant one m-tile ahead, load two ahead
        for mi in range(6, MT):
            if mi + 1 < MT:
                lhs_quant(mi + 1, dve_qc=True)
            if mi + 2 < MT:
                lhs_load_all(mi + 2)
            chain(mi, 0)
            chain(mi, 1)
    return nc


_COMPILED_NC = None


def _get_compiled():
    global _COMPILED_NC
    if _COMPILED_NC is None:
        nc = bacc.Bacc("TRN2", target_bir_lowering=False, debug=False,
                       num_devices=N_CORES)
        build_aqt(nc, M_loc, K, N_loc)
        nc.compile()
        _COMPILED_NC = nc
    return _COMPILED_NC


def _shard(lhs, rhs):
    in_maps = []
    for i in range(N_CORES):
        mg, ng = divmod(i, NG)
        in_maps.append({
            "lhs": np.ascontiguousarray(lhs[mg * M_loc:(mg + 1) * M_loc, :]),
            "rhs": np.ascontiguousarray(rhs[:, ng * N_loc:(ng + 1) * N_loc]),
        })
    return in_maps


def kernel(lhs, rhs, _trace=False, _trace_kwargs=None):
    lhs = np.asarray(lhs, np.float32)
    rhs = np.asarray(rhs, np.float32)
    nc = _get_compiled()
    res = run_bass_kernel_spmd(nc, _shard(lhs, rhs), core_ids=list(range(N_CORES)),
                               trace=_trace, **(_trace_kwargs or {}))
    out = np.empty((M, N), np.float32)
    for i in range(N_CORES):
        mg, ng = divmod(i, NG)
        out[mg * M_loc:(mg + 1) * M_loc, ng * N_loc:(ng + 1) * N_loc] = \
            res.results[i]["out"]
    kernel.last_result = res
    return out
